# revision 2
# baseline (speedup 1.0000x reference)
"""Trainium2 Bass kernel for nn_AttentionBlock (B=2, C=256, D=8, H=32, W=32).

reference math:
    xf = x.reshape(B, C, N)                        # N = 8192
    q = wq @ xf + bq                               # (B, 32, N)
    k = wk @ xf + bk                               # (B, 32, N)
    v = wv @ xf + bv                               # (B, 256, N)
    attn = softmax(q^T k, axis=-1)                 # (B, N, N)
    out = attn @ v^T                               # (B, N, C) buffer
    result = gamma * out.reshape(B, C, d, h, w) + x

Sharding (8 cores): core i -> batch b = i//4, query-chunk c = i%4 of 2048
rows.  Each core gets its batch's full xf (for K/V), a host-sliced xq for
its Q rows, and the matching flat residual slice.  No collectives.

Device algorithm per core (scores are tiny, |S| < ~4, so softmax is computed
without max-subtraction):
    out = (P @ [vT | 1]) ; rows normalized by the appended ones-column
where P = exp(S^T) is materialized in fp8-e4m3.  S^T is computed in bf16
(keys on partitions, queries on free dim) via 4x row-banded K=32 matmuls;
exp alternates between ScalarE (ACTIVATE Exp, fp8 out) and VectorE (a
Schraudolph bit-trick: uint8(S*8*log2e + 56.5) reinterpreted as e4m3).
attn@V and the v-projection run as fp8 DoubleRow matmuls (256-deep
contraction, 2x PE throughput), f32 PSUM accumulation over 32 key-pair
groups.  The epilogue fuses *1/rowsum (ScalarE scale-copy) + residual
(VectorE) into the PSUM copyback.  gamma is folded into wv on the host;
gamma*bv is folded into the residual (bias passes through softmax
averaging unchanged).
"""

import numpy as np

B, C, Dd, Hh, Ww = 2, 256, 8, 32, 32
N = Dd * Hh * Ww          # 8192
CQK = C // 8              # 32
NCORES = 8
QCHUNK = N // 4           # 2048 query rows per core
P = 128

# Schraudolph constants for e4m3 bits: bits = s*8*log2(e) + (7*8 + C_ADJ)
SCH_SCALE = 8.0 / float(np.log(2.0))
SCH_C_ADJ = 0.5           # tuned; robust to floor-vs-round convert


def build_graph(n=N, nq=QCHUNK):
    import concourse.bass as bass
    import concourse.tile as tile
    from concourse import bacc, mybir
    from concourse.bass import ds, ts

    f32 = mybir.dt.float32
    bf16 = mybir.dt.bfloat16
    fp8 = mybir.dt.float8e4
    u8 = mybir.dt.uint8
    AF = mybir.ActivationFunctionType
    ALU = mybir.AluOpType
    DR = mybir.MatmulPerfMode.DoubleRow

    n_t = n // 512            # 16: 512-wide column tiles of xf
    nq_t = nq // 512          # 4:  512-wide column tiles of xq
    m_tiles = n // P          # 64: 128-wide key tiles
    n_grp = m_tiles // 2      # 32: key-pair groups (256 keys)
    n_sc = nq // 512          # 4:  query subchunks
    cin_o = C // P            # 2

    nc = bacc.Bacc()
    xf_d = nc.declare_dram_parameter("xf", [C, n], bf16, isOutput=False)
    xf8_d = nc.declare_dram_parameter("xf8", [C, n], fp8, isOutput=False)
    xq_d = nc.declare_dram_parameter("xq", [C, nq], bf16, isOutput=False)
    xres_d = nc.declare_dram_parameter("xres", [nq, C], f32, isOutput=False)
    wqT_d = nc.declare_dram_parameter("wqT", [C, CQK], bf16, isOutput=False)
    wkT_d = nc.declare_dram_parameter("wkT", [C, CQK], bf16, isOutput=False)
    wvT_d = nc.declare_dram_parameter("wvT", [C, C], fp8, isOutput=False)
    bq_d = nc.declare_dram_parameter("bq", [CQK, 1], f32, isOutput=False)
    bk_d = nc.declare_dram_parameter("bk", [CQK, 1], f32, isOutput=False)
    out_d = nc.declare_dram_parameter("out", [nq, C], f32, isOutput=True)

    with tile.TileContext(nc) as tc:
        with tc.tile_pool(name="singles", bufs=1) as singles, \
             tc.tile_pool(name="ostage", bufs=3) as ostage, \
             tc.tile_pool(name="small", bufs=4) as small, \
             tc.tile_pool(name="ptp", bufs=3) as ptp:

            # ---- constants / weights -------------------------------------
            wqT_s = singles.tile([P, cin_o, CQK], bf16)
            wkT_s = singles.tile([P, cin_o, CQK], bf16)
            wvT_s = singles.tile([P, cin_o, C], fp8)
            for d, sb in ((wqT_d, wqT_s), (wkT_d, wkT_s), (wvT_d, wvT_s)):
                nc.gpsimd.dma_start(out=sb[:], in_=d[:].rearrange(
                    "(co p) m -> p co m", p=P))

            bq_s = singles.tile([P, 1], f32)
            bk_s = singles.tile([P, 1], f32)
            nc.gpsimd.dma_start(out=bq_s[0:CQK, :], in_=bq_d[:])
            nc.gpsimd.dma_start(out=bk_s[0:CQK, :], in_=bk_d[:])

            xres_s = singles.tile([P, nq // P, C], f32)
            nc.gpsimd.dma_start(out=xres_s, in_=xres_d[:].rearrange(
                "(t p) c -> p t c", p=P))

            # ---- load xf (bf16 + fp8), xq (cast on host) -----------------
            xf_bf = singles.tile([P, cin_o, n], bf16)
            xfr = xf_d[:].rearrange("(co p) m -> p co m", p=P)
            for t in range(4):
                nc.gpsimd.dma_start(out=xf_bf[:, :, ts(t, n // 4)],
                                    in_=xfr[:, :, ts(t, n // 4)])
            xf8_s = singles.tile([P, cin_o, n], fp8)
            xf8r = xf8_d[:].rearrange("(co p) m -> p co m", p=P)
            for t in range(2):
                nc.gpsimd.dma_start(out=xf8_s[:, :, ts(t, n // 2)],
                                    in_=xf8r[:, :, ts(t, n // 2)])
            xq_bf = singles.tile([P, cin_o, nq], bf16)
            xqr = xq_d[:].rearrange("(co p) m -> p co m", p=P)
            for t in range(2):
                nc.gpsimd.dma_start(out=xq_bf[:, :, ts(t, nq // 2)],
                                    in_=xqr[:, :, ts(t, nq // 2)])

            # ---- projections ---------------------------------------------
            k_rep = singles.tile([P, n_t, 512], bf16)
            q_rep = singles.tile([P, n_sc, 512], bf16)
            vT = singles.tile([P, m_tiles, C + 1], fp8)
            nc.vector.memset(vT[:, :, C:C + 1], 1.0)

            with tc.tile_pool(name="pp", bufs=2, space="PSUM") as pp:
                # k (all n columns), written to partition group 0 of k_rep
                for t in range(n_t):
                    ps_k = pp.tile([P, 512], f32, tag="psk", name="ps_k")
                    for co in range(cin_o):
                        nc.tensor.matmul(
                            ps_k[0:CQK, :], lhsT=wkT_s[:, co, :],
                            rhs=xf_bf[:, co, ts(t, 512)],
                            start=(co == 0), stop=(co == cin_o - 1))
                    nc.scalar.activation(
                        k_rep[0:CQK, t, :], ps_k[0:CQK, :], AF.Identity,
                        bias=bk_s[0:CQK, :])
                # q (nq columns only)
                for t in range(nq_t):
                    ps_q = pp.tile([P, 512], f32, tag="psk", name="ps_q")
                    for co in range(cin_o):
                        nc.tensor.matmul(
                            ps_q[0:CQK, :], lhsT=wqT_s[:, co, :],
                            rhs=xq_bf[:, co, ts(t, 512)],
                            start=(co == 0), stop=(co == cin_o - 1))
                    nc.scalar.activation(
                        q_rep[0:CQK, t, :], ps_q[0:CQK, :], AF.Identity,
                        bias=bq_s[0:CQK, :])
                # replicate k, q to partition groups 1..3 (SBUF->SBUF DMA)
                for j in range(1, 4):
                    nc.gpsimd.dma_start(out=k_rep[ds(32 * j, 32), :, :],
                                        in_=k_rep[0:32, :, :])
                    nc.gpsimd.dma_start(out=q_rep[ds(32 * j, 32), :, :],
                                        in_=q_rep[0:32, :, :])
                # vT[m, c] = sum_cin xf[cin, m] * wvT[cin, c]  (no bias:
                # gamma*bv is folded into xres on the host).  fp8 DoubleRow
                # contracts both cin halves in one matmul; copies to fp8
                # SBUF alternate ScalarE/VectorE, 2 m-tiles per PSUM pair.
                for mp in range(m_tiles // 2):
                    ps_v = pp.tile([P, 2, C], f32, tag="psv", name="ps_v")
                    for h in range(2):
                        nc.tensor.matmul(
                            ps_v[:, h, :], lhsT=xf8_s[:, :, ts(2 * mp + h, P)],
                            rhs=wvT_s[:], start=True, stop=True,
                            perf_mode=DR)
                    if mp % 2 == 0:
                        nc.scalar.copy(vT[:, ds(2 * mp, 2), 0:C], ps_v)
                    else:
                        nc.vector.tensor_copy(vT[:, ds(2 * mp, 2), 0:C], ps_v)

            # ---- attention ------------------------------------------------
            outr = out_d[:].rearrange("(t p) c -> p t c", p=P)
            with tc.tile_pool(name="stp", bufs=2, space="PSUM") as stp, \
                 tc.tile_pool(name="op", bufs=1, space="PSUM") as op:
                for sc in range(n_sc):
                    out_ps = [op.tile([P, C + 1], f32, tag=f"ops{qt}",
                                      name=f"out_ps{qt}")
                              for qt in range(4)]
                    for gp in range(n_grp):
                        st = stp.tile([P, 2, 512], f32, tag="st", name="st")
                        for jj in range(2):
                            kt = 2 * gp + jj
                            bnd = kt % 4
                            nc.tensor.matmul(
                                st[:, jj, :],
                                lhsT=k_rep[ds(32 * bnd, 32), kt // 4,
                                           ts(kt % 4, P)],
                                rhs=q_rep[ds(32 * bnd, 32), sc, :],
                                start=True, stop=True,
                                tile_position=(32 * bnd, 0))
                        pT = ptp.tile([P, 2, 512], fp8, tag="pt", name="pT")
                        if gp % 2 == 0:
                            nc.scalar.activation(pT[:], st[:], AF.Exp)
                        else:
                            nc.vector.tensor_scalar(
                                out=pT[:].bitcast(u8), in0=st[:],
                                scalar1=SCH_SCALE, scalar2=56.0 + SCH_C_ADJ,
                                op0=ALU.mult, op1=ALU.add)
                        for qt in range(4):
                            nc.tensor.matmul(
                                out_ps[qt],
                                lhsT=pT[:, :, ts(qt, P)],
                                rhs=vT[:, ds(2 * gp, 2), :],
                                start=(gp == 0), stop=(gp == n_grp - 1),
                                perf_mode=DR)
                    # epilogue: out = psum[:, :C] / rowsum + xres
                    for qt in range(4):
                        rec = small.tile([P, 1], f32, tag="rec", name="rec")
                        nc.vector.reciprocal(rec, out_ps[qt][:, C:C + 1])
                        ot = ostage.tile([P, C], f32, tag="ot", name="ot")
                        nc.scalar.activation(ot, out_ps[qt][:, 0:C],
                                             AF.Identity, scale=rec)
                        nc.vector.tensor_add(ot, ot, xres_s[:, 4 * sc + qt, :])
                        nc.sync.dma_start(out=outr[:, 4 * sc + qt, :], in_=ot)
    nc.compile()
    return nc


_nc_cache = {}


def _get_graph(n=N, nq=QCHUNK):
    key = (n, nq)
    if key not in _nc_cache:
        _nc_cache[key] = build_graph(n, nq)
    return _nc_cache[key]


def _make_in_maps(x, wq, bq, wk, bk, wv, bv, gamma, n=N, nq=QCHUNK):
    import ml_dtypes
    bf = ml_dtypes.bfloat16
    e4 = ml_dtypes.float8_e4m3
    xf = np.ascontiguousarray(x.reshape(B, C, n)).astype(np.float32)
    xf16 = xf.astype(bf)
    xf8 = xf.astype(e4)
    g = float(np.asarray(gamma).reshape(-1)[0])
    wqT = np.ascontiguousarray(np.asarray(wq, dtype=np.float32).T).astype(bf)
    wkT = np.ascontiguousarray(np.asarray(wk, dtype=np.float32).T).astype(bf)
    wvT = np.ascontiguousarray(
        (g * np.asarray(wv, dtype=np.float32)).T).astype(e4)
    bq2 = np.asarray(bq, dtype=np.float32).reshape(CQK, 1)
    bk2 = np.asarray(bk, dtype=np.float32).reshape(CQK, 1)
    gbv = (g * np.asarray(bv, dtype=np.float32))[None, :]
    nchunks = n // nq
    in_maps = []
    for i in range(NCORES):
        b, c = divmod(i, nchunks)
        n0 = c * nq
        xres = (xf[b].reshape(-1)[n0 * C:(n0 + nq) * C]
                .reshape(nq, C) + gbv).astype(np.float32)
        in_maps.append({
            "xf": xf16[b],
            "xf8": xf8[b],
            "xq": np.ascontiguousarray(xf16[b][:, n0:n0 + nq]),
            "xres": xres,
            "wqT": wqT, "wkT": wkT, "wvT": wvT,
            "bq": bq2, "bk": bk2,
        })
    return in_maps


def _assemble(results, n=N, nq=QCHUNK):
    nchunks = n // nq
    outs = []
    for b in range(B):
        buf = np.concatenate(
            [results[b * nchunks + c]["out"] for c in range(nchunks)], axis=0)
        outs.append(buf.reshape(C, Dd, Hh, Ww))
    return np.stack(outs).astype(np.float32)


def kernel(x, wq, bq, wk, bk, wv, bv, gamma):
    from concourse.bass_utils import run_bass_kernel_spmd
    nc = _get_graph()
    in_maps = _make_in_maps(x, wq, bq, wk, bk, wv, bv, gamma)
    res = run_bass_kernel_spmd(nc, in_maps, core_ids=list(range(NCORES)))
    return _assemble(res.results)


# revision 5
# speedup vs baseline: 1.1026x; 1.1026x over previous
"""Trainium2 Bass kernel for nn_AttentionBlock (B=2, C=256, D=8, H=32, W=32).

reference math:
    xf = x.reshape(B, C, N)                        # N = 8192
    q = wq @ xf + bq                               # (B, 32, N)
    k = wk @ xf + bk                               # (B, 32, N)
    v = wv @ xf + bv                               # (B, 256, N)
    attn = softmax(q^T k, axis=-1)                 # (B, N, N)
    out = attn @ v^T                               # (B, N, C) buffer
    result = gamma * out.reshape(B, C, d, h, w) + x

Sharding (8 cores): core i -> batch b = i//4, query-chunk c = i%4 of 2048
rows.  Each core gets its batch's full xf (for K/V), a host-sliced xq for
its Q rows, and the matching flat residual slice.  No collectives.

Device algorithm per core (scores are tiny, |S| < ~4, so softmax is computed
without max-subtraction):
    out = (P @ [vT | 1]) ; rows normalized by the appended ones-column
where P = exp(S^T) is materialized in fp8-e4m3.  S^T is computed in bf16
(keys on partitions, queries on free dim) via 4x row-banded K=32 matmuls;
exp alternates between ScalarE (ACTIVATE Exp, fp8 out) and VectorE (a
Schraudolph bit-trick: uint8(S*8*log2e + 56.5) reinterpreted as e4m3).
attn@V and the v-projection run as fp8 DoubleRow matmuls (256-deep
contraction, 2x PE throughput), f32 PSUM accumulation over 32 key-pair
groups.  The epilogue fuses *1/rowsum (ScalarE scale-copy) + residual
(VectorE) into the PSUM copyback.  gamma is folded into wv on the host;
gamma*bv is folded into the residual (bias passes through softmax
averaging unchanged).
"""

import numpy as np

B, C, Dd, Hh, Ww = 2, 256, 8, 32, 32
N = Dd * Hh * Ww          # 8192
CQK = C // 8              # 32
NCORES = 8
QCHUNK = N // 4           # 2048 query rows per core
P = 128

# Schraudolph constants for e4m3 bits: bits = s*8*log2(e) + (7*8 + C_ADJ)
SCH_SCALE = 8.0 / float(np.log(2.0))
SCH_C_ADJ = 0.5           # tuned; robust to floor-vs-round convert


def build_graph(n=N, nq=QCHUNK):
    import concourse.bass as bass
    import concourse.tile as tile
    from concourse import bacc, mybir
    from concourse.bass import ds, ts

    f32 = mybir.dt.float32
    bf16 = mybir.dt.bfloat16
    fp8 = mybir.dt.float8e4
    u8 = mybir.dt.uint8
    AF = mybir.ActivationFunctionType
    ALU = mybir.AluOpType
    DR = mybir.MatmulPerfMode.DoubleRow

    n_t = n // 512            # 16: 512-wide column tiles of xf
    nq_t = nq // 512          # 4:  512-wide column tiles of xq
    m_tiles = n // P          # 64: 128-wide key tiles
    n_grp = m_tiles // 2      # 32: key-pair groups (256 keys)
    n_sc = nq // 512          # 4:  query subchunks
    cin_o = C // P            # 2

    nc = bacc.Bacc()
    xf_d = nc.declare_dram_parameter("xf", [C, n], bf16, isOutput=False)
    xf8_d = nc.declare_dram_parameter("xf8", [C, n], fp8, isOutput=False)
    xq_d = nc.declare_dram_parameter("xq", [C, nq], bf16, isOutput=False)
    xres_d = nc.declare_dram_parameter("xres", [nq, C], f32, isOutput=False)
    wqT_d = nc.declare_dram_parameter("wqT", [C, CQK], bf16, isOutput=False)
    wkT_d = nc.declare_dram_parameter("wkT", [C, CQK], bf16, isOutput=False)
    wvT_d = nc.declare_dram_parameter("wvT", [C, C], fp8, isOutput=False)
    bq_d = nc.declare_dram_parameter("bq", [CQK, 1], f32, isOutput=False)
    bk_d = nc.declare_dram_parameter("bk", [CQK, 1], f32, isOutput=False)
    out_d = nc.declare_dram_parameter("out", [nq, C], f32, isOutput=True)

    with tile.TileContext(nc) as tc:
        with tc.tile_pool(name="singles", bufs=1) as singles, \
             tc.tile_pool(name="ostage", bufs=3) as ostage, \
             tc.tile_pool(name="small", bufs=4) as small, \
             tc.tile_pool(name="ptp", bufs=3) as ptp:

            # ---- constants / weights -------------------------------------
            wqT_s = singles.tile([P, cin_o, CQK], bf16)
            wkT_s = singles.tile([P, cin_o, CQK], bf16)
            wvT_s = singles.tile([P, cin_o, C], fp8)
            for d, sb in ((wqT_d, wqT_s), (wkT_d, wkT_s), (wvT_d, wvT_s)):
                nc.gpsimd.dma_start(out=sb[:], in_=d[:].rearrange(
                    "(co p) m -> p co m", p=P))

            bq_s = singles.tile([P, 1], f32)
            bk_s = singles.tile([P, 1], f32)
            nc.scalar.dma_start(out=bq_s[0:CQK, :], in_=bq_d[:])
            nc.scalar.dma_start(out=bk_s[0:CQK, :], in_=bk_d[:])

            xres_s = singles.tile([P, nq // P, C], f32)
            nc.scalar.dma_start(out=xres_s, in_=xres_d[:].rearrange(
                "(t p) c -> p t c", p=P))

            # ---- load xf (bf16 + fp8), xq (cast on host); spread the
            # loads over several engine queues so the DMAs overlap -------
            xf_bf = singles.tile([P, cin_o, n], bf16)
            xfr = xf_d[:].rearrange("(co p) m -> p co m", p=P)
            for t in range(4):
                eng = nc.gpsimd if t % 2 == 0 else nc.sync
                eng.dma_start(out=xf_bf[:, :, ts(t, n // 4)],
                              in_=xfr[:, :, ts(t, n // 4)])
            xf8_s = singles.tile([P, cin_o, n], fp8)
            xf8r = xf8_d[:].rearrange("(co p) m -> p co m", p=P)
            for t in range(2):
                eng = nc.gpsimd if t % 2 == 0 else nc.sync
                eng.dma_start(out=xf8_s[:, :, ts(t, n // 2)],
                              in_=xf8r[:, :, ts(t, n // 2)])
            xq_bf = singles.tile([P, cin_o, nq], bf16)
            xqr = xq_d[:].rearrange("(co p) m -> p co m", p=P)
            for t in range(2):
                eng = nc.gpsimd if t % 2 == 0 else nc.sync
                eng.dma_start(out=xq_bf[:, :, ts(t, nq // 2)],
                              in_=xqr[:, :, ts(t, nq // 2)])

            # ---- projections ---------------------------------------------
            k_rep = singles.tile([P, n_t, 512], bf16)
            q_rep = singles.tile([P, n_sc, 512], bf16)
            vT = singles.tile([P, m_tiles, C + 1], fp8)
            nc.vector.memset(vT[:, :, C:C + 1], 1.0)

            with tc.tile_pool(name="pp", bufs=2, space="PSUM") as pp:
                # k (all n columns), written to partition group 0 of k_rep
                for t in range(n_t):
                    ps_k = pp.tile([P, 512], f32, tag="psk", name="ps_k")
                    for co in range(cin_o):
                        nc.tensor.matmul(
                            ps_k[0:CQK, :], lhsT=wkT_s[:, co, :],
                            rhs=xf_bf[:, co, ts(t, 512)],
                            start=(co == 0), stop=(co == cin_o - 1))
                    nc.scalar.activation(
                        k_rep[0:CQK, t, :], ps_k[0:CQK, :], AF.Identity,
                        bias=bk_s[0:CQK, :])
                # q (nq columns only)
                for t in range(nq_t):
                    ps_q = pp.tile([P, 512], f32, tag="psk", name="ps_q")
                    for co in range(cin_o):
                        nc.tensor.matmul(
                            ps_q[0:CQK, :], lhsT=wqT_s[:, co, :],
                            rhs=xq_bf[:, co, ts(t, 512)],
                            start=(co == 0), stop=(co == cin_o - 1))
                    nc.scalar.activation(
                        q_rep[0:CQK, t, :], ps_q[0:CQK, :], AF.Identity,
                        bias=bq_s[0:CQK, :])
                # replicate k, q to partition groups 1..3 (SBUF->SBUF DMA)
                for j in range(1, 4):
                    nc.gpsimd.dma_start(out=k_rep[ds(32 * j, 32), :, :],
                                        in_=k_rep[0:32, :, :])
                    nc.gpsimd.dma_start(out=q_rep[ds(32 * j, 32), :, :],
                                        in_=q_rep[0:32, :, :])
                # vT[m, c] = sum_cin xf[cin, m] * wvT[cin, c]  (no bias:
                # gamma*bv is folded into xres on the host).  fp8 DoubleRow
                # contracts both cin halves in one matmul; copies to fp8
                # SBUF alternate ScalarE/VectorE, 2 m-tiles per PSUM pair.
                for mp in range(m_tiles // 2):
                    ps_v = pp.tile([P, 2, C], f32, tag="psv", name="ps_v")
                    for h in range(2):
                        nc.tensor.matmul(
                            ps_v[:, h, :], lhsT=xf8_s[:, :, ts(2 * mp + h, P)],
                            rhs=wvT_s[:], start=True, stop=True,
                            perf_mode=DR)
                    if mp % 2 == 0:
                        nc.scalar.copy(vT[:, ds(2 * mp, 2), 0:C], ps_v)
                    else:
                        nc.vector.tensor_copy(vT[:, ds(2 * mp, 2), 0:C], ps_v)

            # ---- attention ------------------------------------------------
            outr = out_d[:].rearrange("(t p) c -> p t c", p=P)
            with tc.tile_pool(name="stp", bufs=2, space="PSUM") as stp, \
                 tc.tile_pool(name="op", bufs=1, space="PSUM") as op:
                LA = 2    # S-matmul lookahead (groups) for software pipelining
                for sc in range(n_sc):
                    out_ps = [op.tile([P, C + 1], f32, tag=f"ops{qt}",
                                      name=f"out_ps{qt}")
                              for qt in range(4)]
                    sts = {}

                    def emit_s(gp):
                        st = stp.tile([P, 2, 512], f32, tag="st", name="st")
                        sts[gp] = st
                        for jj in range(2):
                            kt = 2 * gp + jj
                            bnd = kt % 4
                            nc.tensor.matmul(
                                st[:, jj, :],
                                lhsT=k_rep[ds(32 * bnd, 32), kt // 4,
                                           ts(kt % 4, P)],
                                rhs=q_rep[ds(32 * bnd, 32), sc, :],
                                start=True, stop=True,
                                tile_position=(32 * bnd, 0))

                    for gp in range(LA):
                        emit_s(gp)
                    for gp in range(n_grp):
                        st = sts.pop(gp)
                        pT = ptp.tile([P, 2, 512], fp8, tag="pt", name="pT")
                        if gp % 2 == 0:
                            nc.scalar.activation(pT[:], st[:], AF.Exp)
                        else:
                            nc.vector.tensor_scalar(
                                out=pT[:].bitcast(u8), in0=st[:],
                                scalar1=SCH_SCALE, scalar2=56.0 + SCH_C_ADJ,
                                op0=ALU.mult, op1=ALU.add)
                        if gp + LA < n_grp:
                            emit_s(gp + LA)
                        for qt in range(4):
                            nc.tensor.matmul(
                                out_ps[qt],
                                lhsT=pT[:, :, ts(qt, P)],
                                rhs=vT[:, ds(2 * gp, 2), :],
                                start=(gp == 0), stop=(gp == n_grp - 1),
                                perf_mode=DR)
                    # epilogue: out = psum[:, :C] / rowsum + xres
                    for qt in range(4):
                        rec = small.tile([P, 1], f32, tag="rec", name="rec")
                        nc.vector.reciprocal(rec, out_ps[qt][:, C:C + 1])
                        ot = ostage.tile([P, C], f32, tag="ot", name="ot")
                        nc.scalar.activation(ot, out_ps[qt][:, 0:C],
                                             AF.Identity, scale=rec)
                        nc.vector.tensor_add(ot, ot, xres_s[:, 4 * sc + qt, :])
                        nc.sync.dma_start(out=outr[:, 4 * sc + qt, :], in_=ot)
    nc.compile()
    return nc


_nc_cache = {}


def _get_graph(n=N, nq=QCHUNK):
    key = (n, nq)
    if key not in _nc_cache:
        _nc_cache[key] = build_graph(n, nq)
    return _nc_cache[key]


def _make_in_maps(x, wq, bq, wk, bk, wv, bv, gamma, n=N, nq=QCHUNK):
    import ml_dtypes
    bf = ml_dtypes.bfloat16
    e4 = ml_dtypes.float8_e4m3
    xf = np.ascontiguousarray(x.reshape(B, C, n)).astype(np.float32)
    xf16 = xf.astype(bf)
    xf8 = xf.astype(e4)
    g = float(np.asarray(gamma).reshape(-1)[0])
    wqT = np.ascontiguousarray(np.asarray(wq, dtype=np.float32).T).astype(bf)
    wkT = np.ascontiguousarray(np.asarray(wk, dtype=np.float32).T).astype(bf)
    wvT = np.ascontiguousarray(
        (g * np.asarray(wv, dtype=np.float32)).T).astype(e4)
    bq2 = np.asarray(bq, dtype=np.float32).reshape(CQK, 1)
    bk2 = np.asarray(bk, dtype=np.float32).reshape(CQK, 1)
    gbv = (g * np.asarray(bv, dtype=np.float32))[None, :]
    nchunks = n // nq
    in_maps = []
    for i in range(NCORES):
        b, c = divmod(i, nchunks)
        n0 = c * nq
        xres = (xf[b].reshape(-1)[n0 * C:(n0 + nq) * C]
                .reshape(nq, C) + gbv).astype(np.float32)
        in_maps.append({
            "xf": xf16[b],
            "xf8": xf8[b],
            "xq": np.ascontiguousarray(xf16[b][:, n0:n0 + nq]),
            "xres": xres,
            "wqT": wqT, "wkT": wkT, "wvT": wvT,
            "bq": bq2, "bk": bk2,
        })
    return in_maps


def _assemble(results, n=N, nq=QCHUNK):
    nchunks = n // nq
    outs = []
    for b in range(B):
        buf = np.concatenate(
            [results[b * nchunks + c]["out"] for c in range(nchunks)], axis=0)
        outs.append(buf.reshape(C, Dd, Hh, Ww))
    return np.stack(outs).astype(np.float32)


def kernel(x, wq, bq, wk, bk, wv, bv, gamma):
    from concourse.bass_utils import run_bass_kernel_spmd
    nc = _get_graph()
    in_maps = _make_in_maps(x, wq, bq, wk, bk, wv, bv, gamma)
    res = run_bass_kernel_spmd(nc, in_maps, core_ids=list(range(NCORES)))
    return _assemble(res.results)


# revision 7
# speedup vs baseline: 1.2981x; 1.1773x over previous
"""Trainium2 Bass kernel for nn_AttentionBlock (B=2, C=256, D=8, H=32, W=32).

reference math:
    xf = x.reshape(B, C, N)                        # N = 8192
    q = wq @ xf + bq                               # (B, 32, N)
    k = wk @ xf + bk                               # (B, 32, N)
    v = wv @ xf + bv                               # (B, 256, N)
    attn = softmax(q^T k, axis=-1)                 # (B, N, N)
    out = attn @ v^T                               # (B, N, C) buffer
    result = gamma * out.reshape(B, C, d, h, w) + x

Sharding (8 cores): core i -> batch b = i//4, query-chunk c = i%4 of 2048
rows.  Each core gets its batch's full xf (for K/V), a host-sliced xq for
its Q rows, and the matching flat residual slice.  No collectives.

Device algorithm per core (scores are tiny, |S| < ~4, so softmax is computed
without max-subtraction):
    out = (P @ [vT | 1]) ; rows normalized by the appended ones-column
where P = exp(S^T) is materialized in fp8-e4m3.  S^T is computed in bf16
(keys on partitions, queries on free dim) via 4x row-banded K=32 matmuls;
exp alternates between ScalarE (ACTIVATE Exp, fp8 out) and VectorE (a
Schraudolph bit-trick: uint8(S*8*log2e + 56.5) reinterpreted as e4m3).
attn@V and the v-projection run as fp8 DoubleRow matmuls (256-deep
contraction, 2x PE throughput), f32 PSUM accumulation over 32 key-pair
groups.  The epilogue fuses *1/rowsum (ScalarE scale-copy) + residual
(VectorE) into the PSUM copyback.  gamma is folded into wv on the host;
gamma*bv is folded into the residual (bias passes through softmax
averaging unchanged).
"""

import numpy as np

B, C, Dd, Hh, Ww = 2, 256, 8, 32, 32
N = Dd * Hh * Ww          # 8192
CQK = C // 8              # 32
NCORES = 8
QCHUNK = N // 4           # 2048 query rows per core
P = 128

# Schraudolph constants for e4m3 bits: bits = s*8*log2(e) + (7*8 + C_ADJ)
SCH_SCALE = 8.0 / float(np.log(2.0))
SCH_C_ADJ = 0.5           # tuned; robust to floor-vs-round convert


def build_graph(n=N, nq=QCHUNK):
    import concourse.bass as bass
    import concourse.tile as tile
    from concourse import bacc, mybir
    from concourse.bass import ds, ts

    f32 = mybir.dt.float32
    bf16 = mybir.dt.bfloat16
    fp8 = mybir.dt.float8e4
    u8 = mybir.dt.uint8
    AF = mybir.ActivationFunctionType
    ALU = mybir.AluOpType
    DR = mybir.MatmulPerfMode.DoubleRow

    n_t = n // 512            # 16: 512-wide column tiles of xf
    nq_t = nq // 512          # 4:  512-wide column tiles of xq
    m_tiles = n // P          # 64: 128-wide key tiles
    n_grp = m_tiles // 2      # 32: key-pair groups (256 keys)
    n_sc = nq // 512          # 4:  query subchunks
    cin_o = C // P            # 2

    nc = bacc.Bacc()
    xf_d = nc.declare_dram_parameter("xf", [C, n], bf16, isOutput=False)
    xf8_d = nc.declare_dram_parameter("xf8", [C, n], fp8, isOutput=False)
    xq_d = nc.declare_dram_parameter("xq", [C, nq], bf16, isOutput=False)
    xres_d = nc.declare_dram_parameter("xres", [nq, C], f32, isOutput=False)
    wqT_d = nc.declare_dram_parameter("wqT", [C, CQK], bf16, isOutput=False)
    wkT_d = nc.declare_dram_parameter("wkT", [C, CQK], bf16, isOutput=False)
    wvT_d = nc.declare_dram_parameter("wvT", [C, C], fp8, isOutput=False)
    bq_d = nc.declare_dram_parameter("bq", [CQK, 1], f32, isOutput=False)
    bk_d = nc.declare_dram_parameter("bk", [CQK, 1], f32, isOutput=False)
    out_d = nc.declare_dram_parameter("out", [nq, C], f32, isOutput=True)

    with tile.TileContext(nc) as tc:
        with tc.tile_pool(name="singles", bufs=1) as singles, \
             tc.tile_pool(name="ostage", bufs=3) as ostage, \
             tc.tile_pool(name="small", bufs=4) as small, \
             tc.tile_pool(name="ptp", bufs=6) as ptp:

            # ---- constants / weights -------------------------------------
            wqT_s = singles.tile([P, cin_o, CQK], bf16)
            wkT_s = singles.tile([P, cin_o, CQK], bf16)
            wvT_s = singles.tile([P, cin_o, C], fp8)
            for d, sb in ((wqT_d, wqT_s), (wkT_d, wkT_s), (wvT_d, wvT_s)):
                nc.gpsimd.dma_start(out=sb[:], in_=d[:].rearrange(
                    "(co p) m -> p co m", p=P))

            bq_s = singles.tile([P, 1], f32)
            bk_s = singles.tile([P, 1], f32)
            nc.scalar.dma_start(out=bq_s[0:CQK, :], in_=bq_d[:])
            nc.scalar.dma_start(out=bk_s[0:CQK, :], in_=bk_d[:])

            xres_s = singles.tile([P, nq // P, C], f32)
            nc.scalar.dma_start(out=xres_s, in_=xres_d[:].rearrange(
                "(t p) c -> p t c", p=P))

            # ---- load xf (bf16 + fp8), xq (cast on host); spread the
            # loads over several engine queues so the DMAs overlap -------
            xf_bf = singles.tile([P, cin_o, n], bf16)
            xfr = xf_d[:].rearrange("(co p) m -> p co m", p=P)
            for t in range(4):
                eng = nc.gpsimd if t % 2 == 0 else nc.sync
                eng.dma_start(out=xf_bf[:, :, ts(t, n // 4)],
                              in_=xfr[:, :, ts(t, n // 4)])
            xf8_s = singles.tile([P, cin_o, n], fp8)
            xf8r = xf8_d[:].rearrange("(co p) m -> p co m", p=P)
            for t in range(2):
                eng = nc.gpsimd if t % 2 == 0 else nc.sync
                eng.dma_start(out=xf8_s[:, :, ts(t, n // 2)],
                              in_=xf8r[:, :, ts(t, n // 2)])
            xq_bf = singles.tile([P, cin_o, nq], bf16)
            xqr = xq_d[:].rearrange("(co p) m -> p co m", p=P)
            for t in range(2):
                eng = nc.gpsimd if t % 2 == 0 else nc.sync
                eng.dma_start(out=xq_bf[:, :, ts(t, nq // 2)],
                              in_=xqr[:, :, ts(t, nq // 2)])

            # ---- projections ---------------------------------------------
            k_rep = singles.tile([P, n_t, 512], bf16)
            q_rep = singles.tile([P, n_sc, 512], bf16)
            vT = singles.tile([P, m_tiles, C + 1], fp8)
            nc.vector.memset(vT[:, :, C:C + 1], 1.0)

            with tc.tile_pool(name="pp", bufs=2, space="PSUM") as pp:
                # k (all n columns), written to partition group 0 of k_rep
                for t in range(n_t):
                    ps_k = pp.tile([P, 512], f32, tag="psk", name="ps_k")
                    for co in range(cin_o):
                        nc.tensor.matmul(
                            ps_k[0:CQK, :], lhsT=wkT_s[:, co, :],
                            rhs=xf_bf[:, co, ts(t, 512)],
                            start=(co == 0), stop=(co == cin_o - 1))
                    nc.scalar.activation(
                        k_rep[0:CQK, t, :], ps_k[0:CQK, :], AF.Identity,
                        bias=bk_s[0:CQK, :])
                # q (nq columns only)
                for t in range(nq_t):
                    ps_q = pp.tile([P, 512], f32, tag="psk", name="ps_q")
                    for co in range(cin_o):
                        nc.tensor.matmul(
                            ps_q[0:CQK, :], lhsT=wqT_s[:, co, :],
                            rhs=xq_bf[:, co, ts(t, 512)],
                            start=(co == 0), stop=(co == cin_o - 1))
                    nc.scalar.activation(
                        q_rep[0:CQK, t, :], ps_q[0:CQK, :], AF.Identity,
                        bias=bq_s[0:CQK, :])
                # replicate k, q to partition groups 1..3 (SBUF->SBUF DMA)
                for j in range(1, 4):
                    nc.gpsimd.dma_start(out=k_rep[ds(32 * j, 32), :, :],
                                        in_=k_rep[0:32, :, :])
                    nc.gpsimd.dma_start(out=q_rep[ds(32 * j, 32), :, :],
                                        in_=q_rep[0:32, :, :])
                # vT[m, c] = sum_cin xf[cin, m] * wvT[cin, c]  (no bias:
                # gamma*bv is folded into xres on the host).  fp8 DoubleRow
                # contracts both cin halves in one matmul; copies to fp8
                # SBUF alternate ScalarE/VectorE, 2 m-tiles per PSUM pair.
                for mp in range(m_tiles // 2):
                    ps_v = pp.tile([P, 2, C], f32, tag="psv", name="ps_v")
                    for h in range(2):
                        nc.tensor.matmul(
                            ps_v[:, h, :], lhsT=xf8_s[:, :, ts(2 * mp + h, P)],
                            rhs=wvT_s[:], start=True, stop=True,
                            perf_mode=DR)
                    if mp % 2 == 0:
                        nc.scalar.copy(vT[:, ds(2 * mp, 2), 0:C], ps_v)
                    else:
                        nc.vector.tensor_copy(vT[:, ds(2 * mp, 2), 0:C], ps_v)

            # ---- attention ------------------------------------------------
            outr = out_d[:].rearrange("(t p) c -> p t c", p=P)
            with tc.tile_pool(name="stp", bufs=4, space="PSUM") as stp, \
                 tc.tile_pool(name="op", bufs=1, space="PSUM") as op:
                LA = 4    # S-matmul lookahead (key tiles) for sw pipelining
                n_kt = 2 * n_grp
                for sc in range(n_sc):
                    out_ps = [op.tile([P, C + 1], f32, tag=f"ops{qt}",
                                      name=f"out_ps{qt}")
                              for qt in range(4)]
                    sts = {}

                    def emit_s(kt):
                        st = stp.tile([P, 512], f32, tag="st", name="st")
                        sts[kt] = st
                        bnd = kt % 4
                        nc.tensor.matmul(
                            st,
                            lhsT=k_rep[ds(32 * bnd, 32), kt // 4,
                                       ts(kt % 4, P)],
                            rhs=q_rep[ds(32 * bnd, 32), sc, :],
                            start=True, stop=True,
                            tile_position=(32 * bnd, 0))

                    for kt in range(LA):
                        emit_s(kt)
                    for gp in range(n_grp):
                        pT = ptp.tile([P, 2, 512], fp8, tag="pt", name="pT")
                        # the pair's two exp halves run CONCURRENTLY on
                        # ScalarE (true exp) and VectorE (Schraudolph bits)
                        st0 = sts.pop(2 * gp)
                        st1 = sts.pop(2 * gp + 1)
                        nc.scalar.activation(pT[:, 0, :], st0, AF.Exp)
                        nc.vector.tensor_scalar(
                            out=pT[:, 1, :].bitcast(u8), in0=st1,
                            scalar1=SCH_SCALE, scalar2=56.0 + SCH_C_ADJ,
                            op0=ALU.mult, op1=ALU.add)
                        for kt in (2 * gp + LA, 2 * gp + 1 + LA):
                            if kt < n_kt:
                                emit_s(kt)
                        for qt in range(4):
                            nc.tensor.matmul(
                                out_ps[qt],
                                lhsT=pT[:, :, ts(qt, P)],
                                rhs=vT[:, ds(2 * gp, 2), :],
                                start=(gp == 0), stop=(gp == n_grp - 1),
                                perf_mode=DR)
                    # epilogue: out = psum[:, :C] / rowsum + xres
                    for qt in range(4):
                        rec = small.tile([P, 1], f32, tag="rec", name="rec")
                        nc.vector.reciprocal(rec, out_ps[qt][:, C:C + 1])
                        ot = ostage.tile([P, C], f32, tag="ot", name="ot")
                        nc.scalar.activation(ot, out_ps[qt][:, 0:C],
                                             AF.Identity, scale=rec)
                        nc.vector.tensor_add(ot, ot, xres_s[:, 4 * sc + qt, :])
                        nc.sync.dma_start(out=outr[:, 4 * sc + qt, :], in_=ot)
    nc.compile()
    return nc


_nc_cache = {}


def _get_graph(n=N, nq=QCHUNK):
    key = (n, nq)
    if key not in _nc_cache:
        _nc_cache[key] = build_graph(n, nq)
    return _nc_cache[key]


def _make_in_maps(x, wq, bq, wk, bk, wv, bv, gamma, n=N, nq=QCHUNK):
    import ml_dtypes
    bf = ml_dtypes.bfloat16
    e4 = ml_dtypes.float8_e4m3
    xf = np.ascontiguousarray(x.reshape(B, C, n)).astype(np.float32)
    xf16 = xf.astype(bf)
    xf8 = xf.astype(e4)
    g = float(np.asarray(gamma).reshape(-1)[0])
    wqT = np.ascontiguousarray(np.asarray(wq, dtype=np.float32).T).astype(bf)
    wkT = np.ascontiguousarray(np.asarray(wk, dtype=np.float32).T).astype(bf)
    wvT = np.ascontiguousarray(
        (g * np.asarray(wv, dtype=np.float32)).T).astype(e4)
    bq2 = np.asarray(bq, dtype=np.float32).reshape(CQK, 1)
    bk2 = np.asarray(bk, dtype=np.float32).reshape(CQK, 1)
    gbv = (g * np.asarray(bv, dtype=np.float32))[None, :]
    nchunks = n // nq
    in_maps = []
    for i in range(NCORES):
        b, c = divmod(i, nchunks)
        n0 = c * nq
        xres = (xf[b].reshape(-1)[n0 * C:(n0 + nq) * C]
                .reshape(nq, C) + gbv).astype(np.float32)
        in_maps.append({
            "xf": xf16[b],
            "xf8": xf8[b],
            "xq": np.ascontiguousarray(xf16[b][:, n0:n0 + nq]),
            "xres": xres,
            "wqT": wqT, "wkT": wkT, "wvT": wvT,
            "bq": bq2, "bk": bk2,
        })
    return in_maps


def _assemble(results, n=N, nq=QCHUNK):
    nchunks = n // nq
    outs = []
    for b in range(B):
        buf = np.concatenate(
            [results[b * nchunks + c]["out"] for c in range(nchunks)], axis=0)
        outs.append(buf.reshape(C, Dd, Hh, Ww))
    return np.stack(outs).astype(np.float32)


def kernel(x, wq, bq, wk, bk, wv, bv, gamma):
    from concourse.bass_utils import run_bass_kernel_spmd
    nc = _get_graph()
    in_maps = _make_in_maps(x, wq, bq, wk, bk, wv, bv, gamma)
    res = run_bass_kernel_spmd(nc, in_maps, core_ids=list(range(NCORES)))
    return _assemble(res.results)


# revision 9
# speedup vs baseline: 1.5059x; 1.1601x over previous
"""Trainium2 Bass kernel for nn_AttentionBlock (B=2, C=256, D=8, H=32, W=32).

reference math:
    xf = x.reshape(B, C, N)                        # N = 8192
    q = wq @ xf + bq                               # (B, 32, N)
    k = wk @ xf + bk                               # (B, 32, N)
    v = wv @ xf + bv                               # (B, 256, N)
    attn = softmax(q^T k, axis=-1)                 # (B, N, N)
    out = attn @ v^T                               # (B, N, C) buffer
    result = gamma * out.reshape(B, C, d, h, w) + x

Sharding (8 cores): core i -> batch b = i//4, query-chunk c = i%4 of 2048
rows.  Each core gets its batch's full xf (for K/V), a host-sliced xq for
its Q rows, and the matching flat residual slice.  No collectives.

Device algorithm per core (scores are tiny, |S| < ~4, so softmax is computed
without max-subtraction):
    out = (P @ [vT | 1]) ; rows normalized by the appended ones-column
where P = exp(S^T) is materialized in fp8-e4m3.  S^T is computed in bf16
(keys on partitions, queries on free dim) via 4x row-banded K=32 matmuls;
exp alternates between ScalarE (ACTIVATE Exp, fp8 out) and VectorE (a
Schraudolph bit-trick: uint8(S*8*log2e + 56.5) reinterpreted as e4m3).
attn@V and the v-projection run as fp8 DoubleRow matmuls (256-deep
contraction, 2x PE throughput), f32 PSUM accumulation over 32 key-pair
groups.  The epilogue fuses *1/rowsum (ScalarE scale-copy) + residual
(VectorE) into the PSUM copyback.  gamma is folded into wv on the host;
gamma*bv is folded into the residual (bias passes through softmax
averaging unchanged).
"""

import numpy as np

B, C, Dd, Hh, Ww = 2, 256, 8, 32, 32
N = Dd * Hh * Ww          # 8192
CQK = C // 8              # 32
NCORES = 8
QCHUNK = N // 4           # 2048 query rows per core
P = 128

# Schraudolph constants for e4m3 bits: bits = s*8*log2(e) + (7*8 + C_ADJ)
SCH_SCALE = 8.0 / float(np.log(2.0))
SCH_C_ADJ = 0.5           # tuned; robust to floor-vs-round convert
SCH_SCALE16 = 128.0 / float(np.log(2.0))
SCH_BIAS16 = 127.0 * 128.0 + 0.5
AV_FP8 = False            # fp8 DoubleRow attn@V (throttle-prone) vs bf16


def build_graph(n=N, nq=QCHUNK):
    import concourse.bass as bass
    import concourse.tile as tile
    from concourse import bacc, mybir
    from concourse.bass import ds, ts

    f32 = mybir.dt.float32
    bf16 = mybir.dt.bfloat16
    fp8 = mybir.dt.float8e4
    u8 = mybir.dt.uint8
    i16 = mybir.dt.int16
    AF = mybir.ActivationFunctionType
    ALU = mybir.AluOpType
    DR = mybir.MatmulPerfMode.DoubleRow

    n_t = n // 512            # 16: 512-wide column tiles of xf
    nq_t = nq // 512          # 4:  512-wide column tiles of xq
    m_tiles = n // P          # 64: 128-wide key tiles
    n_grp = m_tiles // 2      # 32: key-pair groups (256 keys)
    n_sc = nq // 512          # 4:  query subchunks
    cin_o = C // P            # 2

    nc = bacc.Bacc()
    xf_d = nc.declare_dram_parameter("xf", [C, n], bf16, isOutput=False)
    xf8_d = (nc.declare_dram_parameter("xf8", [C, n], fp8, isOutput=False)
             if AV_FP8 else None)
    xq_d = nc.declare_dram_parameter("xq", [C, nq], bf16, isOutput=False)
    xres_d = nc.declare_dram_parameter("xres", [nq, C], f32, isOutput=False)
    wqT_d = nc.declare_dram_parameter("wqT", [C, CQK], bf16, isOutput=False)
    wkT_d = nc.declare_dram_parameter("wkT", [C, CQK], bf16, isOutput=False)
    wvT_d = nc.declare_dram_parameter("wvT", [C, C],
                                      fp8 if AV_FP8 else bf16, isOutput=False)
    bq_d = nc.declare_dram_parameter("bq", [CQK, 1], f32, isOutput=False)
    bk_d = nc.declare_dram_parameter("bk", [CQK, 1], f32, isOutput=False)
    out_d = nc.declare_dram_parameter("out", [nq, C], f32, isOutput=True)

    with tile.TileContext(nc) as tc:
        with tc.tile_pool(name="singles", bufs=1) as singles, \
             tc.tile_pool(name="ostage", bufs=3) as ostage, \
             tc.tile_pool(name="small", bufs=4) as small, \
             tc.tile_pool(name="ptp", bufs=6) as ptp:

            # ---- constants / weights -------------------------------------
            wqT_s = singles.tile([P, cin_o, CQK], bf16)
            wkT_s = singles.tile([P, cin_o, CQK], bf16)
            av_dt = fp8 if AV_FP8 else bf16
            wvT_s = singles.tile([P, cin_o, C], av_dt)
            for d, sb in ((wqT_d, wqT_s), (wkT_d, wkT_s), (wvT_d, wvT_s)):
                nc.gpsimd.dma_start(out=sb[:], in_=d[:].rearrange(
                    "(co p) m -> p co m", p=P))

            bq_s = singles.tile([P, 1], f32)
            bk_s = singles.tile([P, 1], f32)
            nc.scalar.dma_start(out=bq_s[0:CQK, :], in_=bq_d[:])
            nc.scalar.dma_start(out=bk_s[0:CQK, :], in_=bk_d[:])

            xres_s = singles.tile([P, nq // P, C], f32)
            nc.scalar.dma_start(out=xres_s, in_=xres_d[:].rearrange(
                "(t p) c -> p t c", p=P))

            # ---- load xf (bf16 + fp8), xq (cast on host); spread the
            # loads over several engine queues so the DMAs overlap -------
            xf_bf = singles.tile([P, cin_o, n], bf16)
            xfr = xf_d[:].rearrange("(co p) m -> p co m", p=P)
            for t in range(4):
                eng = nc.gpsimd if t % 2 == 0 else nc.sync
                eng.dma_start(out=xf_bf[:, :, ts(t, n // 4)],
                              in_=xfr[:, :, ts(t, n // 4)])
            if AV_FP8:
                xf8_s = singles.tile([P, cin_o, n], fp8)
                xf8r = xf8_d[:].rearrange("(co p) m -> p co m", p=P)
                for t in range(2):
                    eng = nc.gpsimd if t % 2 == 0 else nc.sync
                    eng.dma_start(out=xf8_s[:, :, ts(t, n // 2)],
                                  in_=xf8r[:, :, ts(t, n // 2)])
            xq_bf = singles.tile([P, cin_o, nq], bf16)
            xqr = xq_d[:].rearrange("(co p) m -> p co m", p=P)
            for t in range(2):
                eng = nc.gpsimd if t % 2 == 0 else nc.sync
                eng.dma_start(out=xq_bf[:, :, ts(t, nq // 2)],
                              in_=xqr[:, :, ts(t, nq // 2)])

            # ---- projections ---------------------------------------------
            k_rep = singles.tile([P, n_t, 512], bf16)
            q_rep = singles.tile([P, n_sc, 512], bf16)
            vT = singles.tile([P, m_tiles, C + 1], av_dt)
            nc.vector.memset(vT[:, :, C:C + 1], 1.0)

            with tc.tile_pool(name="pp", bufs=2, space="PSUM") as pp:
                # k (all n columns), written to partition group 0 of k_rep
                for t in range(n_t):
                    ps_k = pp.tile([P, 512], f32, tag="psk", name="ps_k")
                    for co in range(cin_o):
                        nc.tensor.matmul(
                            ps_k[0:CQK, :], lhsT=wkT_s[:, co, :],
                            rhs=xf_bf[:, co, ts(t, 512)],
                            start=(co == 0), stop=(co == cin_o - 1))
                    nc.scalar.activation(
                        k_rep[0:CQK, t, :], ps_k[0:CQK, :], AF.Identity,
                        bias=bk_s[0:CQK, :])
                # q (nq columns only)
                for t in range(nq_t):
                    ps_q = pp.tile([P, 512], f32, tag="psk", name="ps_q")
                    for co in range(cin_o):
                        nc.tensor.matmul(
                            ps_q[0:CQK, :], lhsT=wqT_s[:, co, :],
                            rhs=xq_bf[:, co, ts(t, 512)],
                            start=(co == 0), stop=(co == cin_o - 1))
                    nc.scalar.activation(
                        q_rep[0:CQK, t, :], ps_q[0:CQK, :], AF.Identity,
                        bias=bq_s[0:CQK, :])
                # replicate k, q to partition groups 1..3 (SBUF->SBUF DMA)
                for j in range(1, 4):
                    nc.gpsimd.dma_start(out=k_rep[ds(32 * j, 32), :, :],
                                        in_=k_rep[0:32, :, :])
                    nc.gpsimd.dma_start(out=q_rep[ds(32 * j, 32), :, :],
                                        in_=q_rep[0:32, :, :])
                # vT[m, c] = sum_cin xf[cin, m] * wvT[cin, c]  (no bias:
                # gamma*bv is folded into xres on the host).  fp8 DoubleRow
                # contracts both cin halves in one matmul; copies to fp8
                # SBUF alternate ScalarE/VectorE, 2 m-tiles per PSUM pair.
                for mp in range(m_tiles // 2):
                    ps_v = pp.tile([P, 2, C], f32, tag="psv", name="ps_v")
                    for h in range(2):
                        if AV_FP8:
                            nc.tensor.matmul(
                                ps_v[:, h, :],
                                lhsT=xf8_s[:, :, ts(2 * mp + h, P)],
                                rhs=wvT_s[:], start=True, stop=True,
                                perf_mode=DR)
                        else:
                            for co in range(cin_o):
                                nc.tensor.matmul(
                                    ps_v[:, h, :],
                                    lhsT=xf_bf[:, co, ts(2 * mp + h, P)],
                                    rhs=wvT_s[:, co, :],
                                    start=(co == 0), stop=(co == cin_o - 1))
                    if mp % 2 == 0:
                        nc.scalar.copy(vT[:, ds(2 * mp, 2), 0:C], ps_v)
                    else:
                        nc.vector.tensor_copy(vT[:, ds(2 * mp, 2), 0:C], ps_v)

            # ---- attention ------------------------------------------------
            outr = out_d[:].rearrange("(t p) c -> p t c", p=P)
            with tc.tile_pool(name="stp", bufs=4, space="PSUM") as stp, \
                 tc.tile_pool(name="op", bufs=1, space="PSUM") as op:
                LA = 4    # S-matmul lookahead (key tiles) for sw pipelining
                n_kt = 2 * n_grp
                for sc in range(n_sc):
                    out_ps = [op.tile([P, C + 1], f32, tag=f"ops{qt}",
                                      name=f"out_ps{qt}")
                              for qt in range(4)]
                    sts = {}

                    def emit_s(kt):
                        st = stp.tile([P, 512], f32, tag="st", name="st")
                        sts[kt] = st
                        bnd = kt % 4
                        nc.tensor.matmul(
                            st,
                            lhsT=k_rep[ds(32 * bnd, 32), kt // 4,
                                       ts(kt % 4, P)],
                            rhs=q_rep[ds(32 * bnd, 32), sc, :],
                            start=True, stop=True,
                            tile_position=(32 * bnd, 0))

                    for kt in range(LA):
                        emit_s(kt)
                    for gp in range(n_grp):
                        pT = ptp.tile([P, 2, 512], av_dt, tag="pt",
                                      name="pT")
                        # the pair's two exp halves run CONCURRENTLY on
                        # ScalarE (true exp) and VectorE (Schraudolph bits)
                        st0 = sts.pop(2 * gp)
                        st1 = sts.pop(2 * gp + 1)
                        nc.scalar.activation(pT[:, 0, :], st0, AF.Exp)
                        if AV_FP8:
                            nc.vector.tensor_scalar(
                                out=pT[:, 1, :].bitcast(u8), in0=st1,
                                scalar1=SCH_SCALE, scalar2=56.0 + SCH_C_ADJ,
                                op0=ALU.mult, op1=ALU.add)
                        else:
                            nc.vector.tensor_scalar(
                                out=pT[:, 1, :].bitcast(i16), in0=st1,
                                scalar1=SCH_SCALE16, scalar2=SCH_BIAS16,
                                op0=ALU.mult, op1=ALU.add)
                        for kt in (2 * gp + LA, 2 * gp + 1 + LA):
                            if kt < n_kt:
                                emit_s(kt)
                        for qt in range(4):
                            if AV_FP8:
                                nc.tensor.matmul(
                                    out_ps[qt],
                                    lhsT=pT[:, :, ts(qt, P)],
                                    rhs=vT[:, ds(2 * gp, 2), :],
                                    start=(gp == 0),
                                    stop=(gp == n_grp - 1),
                                    perf_mode=DR)
                            else:
                                for jj in range(2):
                                    nc.tensor.matmul(
                                        out_ps[qt],
                                        lhsT=pT[:, jj, ts(qt, P)],
                                        rhs=vT[:, 2 * gp + jj, :],
                                        start=(gp == 0 and jj == 0),
                                        stop=(gp == n_grp - 1 and jj == 1))
                    # epilogue: out = psum[:, :C] / rowsum + xres
                    for qt in range(4):
                        rec = small.tile([P, 1], f32, tag="rec", name="rec")
                        nc.vector.reciprocal(rec, out_ps[qt][:, C:C + 1])
                        ot = ostage.tile([P, C], f32, tag="ot", name="ot")
                        nc.scalar.activation(ot, out_ps[qt][:, 0:C],
                                             AF.Identity, scale=rec)
                        nc.vector.tensor_add(ot, ot, xres_s[:, 4 * sc + qt, :])
                        nc.sync.dma_start(out=outr[:, 4 * sc + qt, :], in_=ot)
    nc.compile()
    return nc


_nc_cache = {}


def _get_graph(n=N, nq=QCHUNK):
    key = (n, nq)
    if key not in _nc_cache:
        _nc_cache[key] = build_graph(n, nq)
    return _nc_cache[key]


def _make_in_maps(x, wq, bq, wk, bk, wv, bv, gamma, n=N, nq=QCHUNK):
    import ml_dtypes
    bf = ml_dtypes.bfloat16
    e4 = ml_dtypes.float8_e4m3
    xf = np.ascontiguousarray(x.reshape(B, C, n)).astype(np.float32)
    xf16 = xf.astype(bf)
    xf8 = xf.astype(e4)
    g = float(np.asarray(gamma).reshape(-1)[0])
    wqT = np.ascontiguousarray(np.asarray(wq, dtype=np.float32).T).astype(bf)
    wkT = np.ascontiguousarray(np.asarray(wk, dtype=np.float32).T).astype(bf)
    wvT = np.ascontiguousarray(
        (g * np.asarray(wv, dtype=np.float32)).T).astype(e4 if AV_FP8 else bf)
    bq2 = np.asarray(bq, dtype=np.float32).reshape(CQK, 1)
    bk2 = np.asarray(bk, dtype=np.float32).reshape(CQK, 1)
    gbv = (g * np.asarray(bv, dtype=np.float32))[None, :]
    nchunks = n // nq
    in_maps = []
    for i in range(NCORES):
        b, c = divmod(i, nchunks)
        n0 = c * nq
        xres = (xf[b].reshape(-1)[n0 * C:(n0 + nq) * C]
                .reshape(nq, C) + gbv).astype(np.float32)
        im = {
            "xf": xf16[b],
            "xq": np.ascontiguousarray(xf16[b][:, n0:n0 + nq]),
            "xres": xres,
            "wqT": wqT, "wkT": wkT, "wvT": wvT,
            "bq": bq2, "bk": bk2,
        }
        if AV_FP8:
            im["xf8"] = xf8[b]
        in_maps.append(im)
    return in_maps


def _assemble(results, n=N, nq=QCHUNK):
    nchunks = n // nq
    outs = []
    for b in range(B):
        buf = np.concatenate(
            [results[b * nchunks + c]["out"] for c in range(nchunks)], axis=0)
        outs.append(buf.reshape(C, Dd, Hh, Ww))
    return np.stack(outs).astype(np.float32)


def kernel(x, wq, bq, wk, bk, wv, bv, gamma):
    from concourse.bass_utils import run_bass_kernel_spmd
    nc = _get_graph()
    in_maps = _make_in_maps(x, wq, bq, wk, bk, wv, bv, gamma)
    res = run_bass_kernel_spmd(nc, in_maps, core_ids=list(range(NCORES)))
    return _assemble(res.results)


# revision 12
# speedup vs baseline: 1.5843x; 1.0521x over previous
"""Trainium2 Bass kernel for nn_AttentionBlock (B=2, C=256, D=8, H=32, W=32).

reference math:
    xf = x.reshape(B, C, N)                        # N = 8192
    q = wq @ xf + bq                               # (B, 32, N)
    k = wk @ xf + bk                               # (B, 32, N)
    v = wv @ xf + bv                               # (B, 256, N)
    attn = softmax(q^T k, axis=-1)                 # (B, N, N)
    out = attn @ v^T                               # (B, N, C) buffer
    result = gamma * out.reshape(B, C, d, h, w) + x

Sharding (8 cores): core i -> batch b = i//4, query-chunk c = i%4 of 2048
rows.  Each core gets its batch's full xf (for K/V), a host-sliced xq for
its Q rows, and the matching flat residual slice.  No collectives.

Device algorithm per core (scores are tiny, |S| < ~4, so softmax is computed
without max-subtraction):
    out = (P @ [vT | 1]) ; rows normalized by the appended ones-column
where P = exp(S^T) is materialized in fp8-e4m3.  S^T is computed in bf16
(keys on partitions, queries on free dim) via 4x row-banded K=32 matmuls;
exp alternates between ScalarE (ACTIVATE Exp, fp8 out) and VectorE (a
Schraudolph bit-trick: uint8(S*8*log2e + 56.5) reinterpreted as e4m3).
attn@V and the v-projection run as fp8 DoubleRow matmuls (256-deep
contraction, 2x PE throughput), f32 PSUM accumulation over 32 key-pair
groups.  The epilogue fuses *1/rowsum (ScalarE scale-copy) + residual
(VectorE) into the PSUM copyback.  gamma is folded into wv on the host;
gamma*bv is folded into the residual (bias passes through softmax
averaging unchanged).
"""

import numpy as np

B, C, Dd, Hh, Ww = 2, 256, 8, 32, 32
N = Dd * Hh * Ww          # 8192
CQK = C // 8              # 32
NCORES = 8
QCHUNK = N // 4           # 2048 query rows per core
P = 128

# Schraudolph constants for e4m3 bits: bits = s*8*log2(e) + (7*8 + C_ADJ)
SCH_SCALE = 8.0 / float(np.log(2.0))
SCH_C_ADJ = 0.5           # tuned; robust to floor-vs-round convert
SCH_SCALE16 = 128.0 / float(np.log(2.0))
SCH_BIAS16 = 127.0 * 128.0 + 0.5
AV_FP8 = False            # all-fp8 DoubleRow attn@V (throttle-prone)
DR_MIX = True             # alternate fp8-DR / bf16 groups (keeps HAM warm)


def build_graph(n=N, nq=QCHUNK):
    import concourse.bass as bass
    import concourse.tile as tile
    from concourse import bacc, mybir
    from concourse.bass import ds, ts

    f32 = mybir.dt.float32
    bf16 = mybir.dt.bfloat16
    fp8 = mybir.dt.float8e4
    u8 = mybir.dt.uint8
    i16 = mybir.dt.int16
    AF = mybir.ActivationFunctionType
    ALU = mybir.AluOpType
    DR = mybir.MatmulPerfMode.DoubleRow

    n_t = n // 512            # 16: 512-wide column tiles of xf
    nq_t = nq // 512          # 4:  512-wide column tiles of xq
    m_tiles = n // P          # 64: 128-wide key tiles
    n_grp = m_tiles // 2      # 32: key-pair groups (256 keys)
    n_sc = nq // 512          # 4:  query subchunks
    cin_o = C // P            # 2

    nc = bacc.Bacc()
    xf_d = nc.declare_dram_parameter("xf", [C, n], bf16, isOutput=False)
    use8 = AV_FP8 or DR_MIX
    xf8_d = (nc.declare_dram_parameter("xf8", [C, n], fp8, isOutput=False)
             if use8 else None)
    xq_d = nc.declare_dram_parameter("xq", [C, nq], bf16, isOutput=False)
    xres_d = nc.declare_dram_parameter("xres", [nq, C], f32, isOutput=False)
    wqT_d = nc.declare_dram_parameter("wqT", [C, CQK], bf16, isOutput=False)
    wkT_d = nc.declare_dram_parameter("wkT", [C, CQK], bf16, isOutput=False)
    wvT_d = (None if AV_FP8 else
             nc.declare_dram_parameter("wvT", [C, C], bf16, isOutput=False))
    wvT8_d = (nc.declare_dram_parameter("wvT8", [C, C], fp8, isOutput=False)
              if use8 else None)
    bq_d = nc.declare_dram_parameter("bq", [CQK, 1], f32, isOutput=False)
    bk_d = nc.declare_dram_parameter("bk", [CQK, 1], f32, isOutput=False)
    out_d = nc.declare_dram_parameter("out", [nq, C], f32, isOutput=True)

    with tile.TileContext(nc) as tc:
        with tc.tile_pool(name="singles", bufs=1) as singles, \
             tc.tile_pool(name="ostage", bufs=3) as ostage, \
             tc.tile_pool(name="small", bufs=4) as small, \
             tc.tile_pool(name="ptp", bufs=6) as ptp:

            # ---- constants / weights -------------------------------------
            wqT_s = singles.tile([P, cin_o, CQK], bf16)
            wkT_s = singles.tile([P, cin_o, CQK], bf16)
            wqT_s = singles.tile([P, cin_o, CQK], bf16)
            wkT_s = singles.tile([P, cin_o, CQK], bf16)
            loads = [(wqT_d, wqT_s), (wkT_d, wkT_s)]
            wvT_s = wvT8_s = None
            if not AV_FP8:
                wvT_s = singles.tile([P, cin_o, C], bf16)
                loads.append((wvT_d, wvT_s))
            if use8:
                wvT8_s = singles.tile([P, cin_o, C], fp8)
                loads.append((wvT8_d, wvT8_s))
            for d, sb in loads:
                nc.gpsimd.dma_start(out=sb[:], in_=d[:].rearrange(
                    "(co p) m -> p co m", p=P))

            bq_s = singles.tile([P, 1], f32)
            bk_s = singles.tile([P, 1], f32)
            nc.scalar.dma_start(out=bq_s[0:CQK, :], in_=bq_d[:])
            nc.scalar.dma_start(out=bk_s[0:CQK, :], in_=bk_d[:])

            xres_s = singles.tile([P, nq // P, C], f32)
            nc.scalar.dma_start(out=xres_s, in_=xres_d[:].rearrange(
                "(t p) c -> p t c", p=P))

            # ---- load xf (bf16 + fp8), xq (cast on host); spread the
            # loads over several engine queues so the DMAs overlap -------
            xf_bf = singles.tile([P, cin_o, n], bf16)
            xfr = xf_d[:].rearrange("(co p) m -> p co m", p=P)
            for t in range(4):
                eng = nc.gpsimd if t % 2 == 0 else nc.sync
                eng.dma_start(out=xf_bf[:, :, ts(t, n // 4)],
                              in_=xfr[:, :, ts(t, n // 4)])
            if use8:
                xf8_s = singles.tile([P, cin_o, n], fp8)
                xf8r = xf8_d[:].rearrange("(co p) m -> p co m", p=P)
                for t in range(2):
                    eng = nc.gpsimd if t % 2 == 0 else nc.sync
                    eng.dma_start(out=xf8_s[:, :, ts(t, n // 2)],
                                  in_=xf8r[:, :, ts(t, n // 2)])
            xq_bf = singles.tile([P, cin_o, nq], bf16)
            xqr = xq_d[:].rearrange("(co p) m -> p co m", p=P)
            for t in range(2):
                eng = nc.gpsimd if t % 2 == 0 else nc.sync
                eng.dma_start(out=xq_bf[:, :, ts(t, nq // 2)],
                              in_=xqr[:, :, ts(t, nq // 2)])

            # ---- projections ---------------------------------------------
            k_rep = singles.tile([P, n_t, 512], bf16)
            q_rep = singles.tile([P, n_sc, 512], bf16)
            def dr_of(gp):
                return AV_FP8 or (DR_MIX and gp % 2 == 0)
            vT = vT8 = None
            if not AV_FP8:
                vT = singles.tile([P, m_tiles, C + 1], bf16)
                nc.vector.memset(vT[:, :, C:C + 1], 1.0)
            if use8:
                vT8 = singles.tile([P, m_tiles, C + 1], fp8)
                nc.vector.memset(vT8[:, :, C:C + 1], 1.0)

            with tc.tile_pool(name="pp", bufs=2, space="PSUM") as pp:
                # k (all n columns), written to partition group 0 of k_rep
                for t in range(n_t):
                    ps_k = pp.tile([P, 512], f32, tag="psk", name="ps_k")
                    for co in range(cin_o):
                        nc.tensor.matmul(
                            ps_k[0:CQK, :], lhsT=wkT_s[:, co, :],
                            rhs=xf_bf[:, co, ts(t, 512)],
                            start=(co == 0), stop=(co == cin_o - 1))
                    nc.scalar.activation(
                        k_rep[0:CQK, t, :], ps_k[0:CQK, :], AF.Identity,
                        bias=bk_s[0:CQK, :])
                # q (nq columns only)
                for t in range(nq_t):
                    ps_q = pp.tile([P, 512], f32, tag="psk", name="ps_q")
                    for co in range(cin_o):
                        nc.tensor.matmul(
                            ps_q[0:CQK, :], lhsT=wqT_s[:, co, :],
                            rhs=xq_bf[:, co, ts(t, 512)],
                            start=(co == 0), stop=(co == cin_o - 1))
                    nc.scalar.activation(
                        q_rep[0:CQK, t, :], ps_q[0:CQK, :], AF.Identity,
                        bias=bq_s[0:CQK, :])
                # replicate k, q to partition groups 1..3 (SBUF->SBUF DMA)
                for j in range(1, 4):
                    nc.gpsimd.dma_start(out=k_rep[ds(32 * j, 32), :, :],
                                        in_=k_rep[0:32, :, :])
                    nc.gpsimd.dma_start(out=q_rep[ds(32 * j, 32), :, :],
                                        in_=q_rep[0:32, :, :])
                # vT[m, c] = sum_cin xf[cin, m] * wvT[cin, c]  (no bias:
                # gamma*bv is folded into xres on the host).  fp8 DoubleRow
                # contracts both cin halves in one matmul; copies to fp8
                # SBUF alternate ScalarE/VectorE, 2 m-tiles per PSUM pair.
                for mp in range(m_tiles // 2):
                    ps_v = pp.tile([P, 2, C], f32, tag="psv", name="ps_v")
                    dst = vT8 if dr_of(mp) else vT
                    for h in range(2):
                        if dr_of(mp):
                            nc.tensor.matmul(
                                ps_v[:, h, :],
                                lhsT=xf8_s[:, :, ts(2 * mp + h, P)],
                                rhs=wvT8_s[:], start=True, stop=True,
                                perf_mode=DR)
                        else:
                            for co in range(cin_o):
                                nc.tensor.matmul(
                                    ps_v[:, h, :],
                                    lhsT=xf_bf[:, co, ts(2 * mp + h, P)],
                                    rhs=wvT_s[:, co, :],
                                    start=(co == 0), stop=(co == cin_o - 1))
                    if mp % 2 == 0:
                        nc.scalar.copy(dst[:, ds(2 * mp, 2), 0:C], ps_v)
                    else:
                        nc.vector.tensor_copy(dst[:, ds(2 * mp, 2), 0:C], ps_v)

            # ---- attention ------------------------------------------------
            outr = out_d[:].rearrange("(t p) c -> p t c", p=P)
            with tc.tile_pool(name="stp", bufs=4, space="PSUM") as stp, \
                 tc.tile_pool(name="op", bufs=1, space="PSUM") as op:
                LA = 4    # S-matmul lookahead (key tiles) for sw pipelining
                n_kt = 2 * n_grp
                for sc in range(n_sc):
                    out_ps = [op.tile([P, C + 1], f32, tag=f"ops{qt}",
                                      name=f"out_ps{qt}")
                              for qt in range(4)]
                    sts = {}

                    def emit_s(kt):
                        st = stp.tile([P, 512], f32, tag="st", name="st")
                        sts[kt] = st
                        bnd = kt % 4
                        nc.tensor.matmul(
                            st,
                            lhsT=k_rep[ds(32 * bnd, 32), kt // 4,
                                       ts(kt % 4, P)],
                            rhs=q_rep[ds(32 * bnd, 32), sc, :],
                            start=True, stop=True,
                            tile_position=(32 * bnd, 0))

                    for kt in range(LA):
                        emit_s(kt)
                    for gp in range(n_grp):
                        dr = dr_of(gp)
                        pT = ptp.tile([P, 2, 512], fp8 if dr else bf16,
                                      tag="pt8" if dr else "pt16",
                                      name="pT", bufs=3)
                        # the pair's two exp halves run CONCURRENTLY on
                        # ScalarE (true exp) and VectorE (Schraudolph bits)
                        st0 = sts.pop(2 * gp)
                        st1 = sts.pop(2 * gp + 1)
                        nc.scalar.activation(pT[:, 0, :], st0, AF.Exp)
                        if dr:
                            nc.vector.tensor_scalar(
                                out=pT[:, 1, :].bitcast(u8), in0=st1,
                                scalar1=SCH_SCALE, scalar2=56.0 + SCH_C_ADJ,
                                op0=ALU.mult, op1=ALU.add)
                        else:
                            nc.vector.tensor_scalar(
                                out=pT[:, 1, :].bitcast(i16), in0=st1,
                                scalar1=SCH_SCALE16, scalar2=SCH_BIAS16,
                                op0=ALU.mult, op1=ALU.add)
                        for kt in (2 * gp + LA, 2 * gp + 1 + LA):
                            if kt < n_kt:
                                emit_s(kt)
                        for qt in range(4):
                            if dr:
                                nc.tensor.matmul(
                                    out_ps[qt],
                                    lhsT=pT[:, :, ts(qt, P)],
                                    rhs=vT8[:, ds(2 * gp, 2), :],
                                    start=(gp == 0),
                                    stop=(gp == n_grp - 1),
                                    perf_mode=DR)
                            else:
                                for jj in range(2):
                                    nc.tensor.matmul(
                                        out_ps[qt],
                                        lhsT=pT[:, jj, ts(qt, P)],
                                        rhs=vT[:, 2 * gp + jj, :],
                                        start=(gp == 0 and jj == 0),
                                        stop=(gp == n_grp - 1 and jj == 1))
                    # epilogue: out = psum[:, :C] / rowsum + xres
                    for qt in range(4):
                        rec = small.tile([P, 1], f32, tag="rec", name="rec")
                        nc.vector.reciprocal(rec, out_ps[qt][:, C:C + 1])
                        ot = ostage.tile([P, C], f32, tag="ot", name="ot")
                        nc.scalar.activation(ot, out_ps[qt][:, 0:C],
                                             AF.Identity, scale=rec)
                        nc.vector.tensor_add(ot, ot, xres_s[:, 4 * sc + qt, :])
                        nc.sync.dma_start(out=outr[:, 4 * sc + qt, :], in_=ot)
    nc.compile()
    return nc


_nc_cache = {}


def _get_graph(n=N, nq=QCHUNK):
    key = (n, nq)
    if key not in _nc_cache:
        _nc_cache[key] = build_graph(n, nq)
    return _nc_cache[key]


def _make_in_maps(x, wq, bq, wk, bk, wv, bv, gamma, n=N, nq=QCHUNK):
    import ml_dtypes
    bf = ml_dtypes.bfloat16
    e4 = ml_dtypes.float8_e4m3
    xf = np.ascontiguousarray(x.reshape(B, C, n)).astype(np.float32)
    xf16 = xf.astype(bf)
    xf8 = xf.astype(e4)
    g = float(np.asarray(gamma).reshape(-1)[0])
    wqT = np.ascontiguousarray(np.asarray(wq, dtype=np.float32).T).astype(bf)
    wkT = np.ascontiguousarray(np.asarray(wk, dtype=np.float32).T).astype(bf)
    wvTf = np.ascontiguousarray((g * np.asarray(wv, dtype=np.float32)).T)
    wvT = wvTf.astype(bf)
    wvT8 = wvTf.astype(e4)
    bq2 = np.asarray(bq, dtype=np.float32).reshape(CQK, 1)
    bk2 = np.asarray(bk, dtype=np.float32).reshape(CQK, 1)
    gbv = (g * np.asarray(bv, dtype=np.float32))[None, :]
    nchunks = n // nq
    in_maps = []
    for i in range(NCORES):
        b, c = divmod(i, nchunks)
        n0 = c * nq
        xres = (xf[b].reshape(-1)[n0 * C:(n0 + nq) * C]
                .reshape(nq, C) + gbv).astype(np.float32)
        im = {
            "xf": xf16[b],
            "xq": np.ascontiguousarray(xf16[b][:, n0:n0 + nq]),
            "xres": xres,
            "wqT": wqT, "wkT": wkT,
            "bq": bq2, "bk": bk2,
        }
        if not AV_FP8:
            im["wvT"] = wvT
        if AV_FP8 or DR_MIX:
            im["xf8"] = xf8[b]
            im["wvT8"] = wvT8
        in_maps.append(im)
    return in_maps


def _assemble(results, n=N, nq=QCHUNK):
    nchunks = n // nq
    outs = []
    for b in range(B):
        buf = np.concatenate(
            [results[b * nchunks + c]["out"] for c in range(nchunks)], axis=0)
        outs.append(buf.reshape(C, Dd, Hh, Ww))
    return np.stack(outs).astype(np.float32)


def kernel(x, wq, bq, wk, bk, wv, bv, gamma):
    from concourse.bass_utils import run_bass_kernel_spmd
    nc = _get_graph()
    in_maps = _make_in_maps(x, wq, bq, wk, bk, wv, bv, gamma)
    res = run_bass_kernel_spmd(nc, in_maps, core_ids=list(range(NCORES)))
    return _assemble(res.results)


# revision 16
# speedup vs baseline: 1.6880x; 1.0654x over previous
"""Trainium2 Bass kernel for nn_AttentionBlock (B=2, C=256, D=8, H=32, W=32).

reference math:
    xf = x.reshape(B, C, N)                        # N = 8192
    q = wq @ xf + bq                               # (B, 32, N)
    k = wk @ xf + bk                               # (B, 32, N)
    v = wv @ xf + bv                               # (B, 256, N)
    attn = softmax(q^T k, axis=-1)                 # (B, N, N)
    out = attn @ v^T                               # (B, N, C) buffer
    result = gamma * out.reshape(B, C, d, h, w) + x

Sharding (8 cores): core i -> batch b = i//4, query-chunk c = i%4 of 2048
rows.  Each core gets its batch's full xf (for K/V), a host-sliced xq for
its Q rows, and the matching flat residual slice.  No collectives.

Device algorithm per core (scores are tiny, |S| < ~4, so softmax is computed
without max-subtraction):
    out = (P @ [vT | 1]) ; rows normalized by the appended ones-column
where P = exp(S^T) is materialized in fp8-e4m3.  S^T is computed in bf16
(keys on partitions, queries on free dim) via 4x row-banded K=32 matmuls;
exp alternates between ScalarE (ACTIVATE Exp, fp8 out) and VectorE (a
Schraudolph bit-trick: uint8(S*8*log2e + 56.5) reinterpreted as e4m3).
attn@V and the v-projection run as fp8 DoubleRow matmuls (256-deep
contraction, 2x PE throughput), f32 PSUM accumulation over 32 key-pair
groups.  The epilogue fuses *1/rowsum (ScalarE scale-copy) + residual
(VectorE) into the PSUM copyback.  gamma is folded into wv on the host;
gamma*bv is folded into the residual (bias passes through softmax
averaging unchanged).
"""

import numpy as np

B, C, Dd, Hh, Ww = 2, 256, 8, 32, 32
N = Dd * Hh * Ww          # 8192
CQK = C // 8              # 32
NCORES = 8
QCHUNK = N // 4           # 2048 query rows per core
P = 128

# Schraudolph constants for e4m3 bits: bits = s*8*log2(e) + (7*8 + C_ADJ)
SCH_SCALE = 8.0 / float(np.log(2.0))
SCH_C_ADJ = 0.5           # tuned; robust to floor-vs-round convert
SCH_SCALE16 = 128.0 / float(np.log(2.0))
SCH_BIAS16 = 127.0 * 128.0 + 0.5
AV_FP8 = False            # all-fp8 DoubleRow attn@V (throttle-prone)
DR_MIX = True             # alternate fp8-DR / bf16 groups (keeps HAM warm)


def build_graph(n=N, nq=QCHUNK):
    import concourse.bass as bass
    import concourse.tile as tile
    from concourse import bacc, mybir
    from concourse.bass import ds, ts

    f32 = mybir.dt.float32
    bf16 = mybir.dt.bfloat16
    fp8 = mybir.dt.float8e4
    u8 = mybir.dt.uint8
    i16 = mybir.dt.int16
    AF = mybir.ActivationFunctionType
    ALU = mybir.AluOpType
    DR = mybir.MatmulPerfMode.DoubleRow

    n_t = n // 512            # 16: 512-wide column tiles of xf
    nq_t = nq // 512          # 4:  512-wide column tiles of xq
    m_tiles = n // P          # 64: 128-wide key tiles
    n_grp = m_tiles // 2      # 32: key-pair groups (256 keys)
    n_sc = nq // 512          # 4:  query subchunks
    cin_o = C // P            # 2

    nc = bacc.Bacc()
    xf_d = nc.declare_dram_parameter("xf", [C, n], bf16, isOutput=False)
    use8 = AV_FP8 or DR_MIX
    xf8_d = (nc.declare_dram_parameter("xf8", [C, n], fp8, isOutput=False)
             if use8 else None)
    xq_d = nc.declare_dram_parameter("xq", [C, nq], bf16, isOutput=False)
    xres_d = nc.declare_dram_parameter("xres", [nq, C], f32, isOutput=False)
    wqT_d = nc.declare_dram_parameter("wqT", [C, CQK], bf16, isOutput=False)
    wkT_d = nc.declare_dram_parameter("wkT", [C, CQK], bf16, isOutput=False)
    wvT_d = (None if AV_FP8 else
             nc.declare_dram_parameter("wvT", [C, C], bf16, isOutput=False))
    wvT8_d = (nc.declare_dram_parameter("wvT8", [C, C], fp8, isOutput=False)
              if use8 else None)
    bq_d = nc.declare_dram_parameter("bq", [CQK, 1], f32, isOutput=False)
    bk_d = nc.declare_dram_parameter("bk", [CQK, 1], f32, isOutput=False)
    out_d = nc.declare_dram_parameter("out", [nq, C], f32, isOutput=True)

    with tile.TileContext(nc) as tc:
        with tc.tile_pool(name="singles", bufs=1) as singles, \
             tc.tile_pool(name="ostage", bufs=3) as ostage, \
             tc.tile_pool(name="small", bufs=4) as small, \
             tc.tile_pool(name="ptp", bufs=6) as ptp:

            # ---- constants / weights -------------------------------------
            wqT_s = singles.tile([P, cin_o, CQK], bf16)
            wkT_s = singles.tile([P, cin_o, CQK], bf16)
            loads = [(wqT_d, wqT_s), (wkT_d, wkT_s)]
            wvT_s = wvT8_s = None
            if not AV_FP8:
                wvT_s = singles.tile([P, cin_o, C], bf16)
                loads.append((wvT_d, wvT_s))
            if use8:
                wvT8_s = singles.tile([P, cin_o, C], fp8)
                loads.append((wvT8_d, wvT8_s))
            for d, sb in loads:
                nc.scalar.dma_start(out=sb[:], in_=d[:].rearrange(
                    "(co p) m -> p co m", p=P))

            bq_s = singles.tile([P, 1], f32)
            bk_s = singles.tile([P, 1], f32)
            nc.scalar.dma_start(out=bq_s[0:CQK, :], in_=bq_d[:])
            nc.scalar.dma_start(out=bk_s[0:CQK, :], in_=bk_d[:])

            xres_s = singles.tile([P, nq // P, C], f32)
            nc.scalar.dma_start(out=xres_s, in_=xres_d[:].rearrange(
                "(t p) c -> p t c", p=P))

            # ---- input staging buffers (loads chunked + interleaved with
            # the projection matmuls below so the PE starts early) --------
            xf_bf = singles.tile([P, cin_o, n], bf16)
            xfr = xf_d[:].rearrange("(co p) m -> p co m", p=P)
            if use8:
                xf8_s = singles.tile([P, cin_o, n], fp8)
                xf8r = xf8_d[:].rearrange("(co p) m -> p co m", p=P)
            xq_bf = singles.tile([P, cin_o, nq], bf16)
            xqr = xq_d[:].rearrange("(co p) m -> p co m", p=P)

            # ---- projections ---------------------------------------------
            k_rep = singles.tile([P, n_t, 512], bf16)
            q_rep = singles.tile([P, n_sc, 512], bf16)
            def dr_of(gp):
                return AV_FP8 or (DR_MIX and gp % 4 != 3)
            vT = vT8 = None
            if not AV_FP8:
                vT = singles.tile([P, m_tiles, C + 1], bf16)
                nc.vector.memset(vT[:, :, C:C + 1], 1.0)
            if use8:
                vT8 = singles.tile([P, m_tiles, C + 1], fp8)
                nc.vector.memset(vT8[:, :, C:C + 1], 1.0)

            with tc.tile_pool(name="pp", bufs=2, space="PSUM") as pp:
                # k (all n columns), written to partition group 0 of k_rep.
                # The xf chunk load for tile t is issued right before the
                # matmuls that consume it, alternating DMA queues.
                for t in range(n_t):
                    eng = nc.gpsimd if t % 2 == 0 else nc.sync
                    eng.dma_start(out=xf_bf[:, :, ts(t, 512)],
                                  in_=xfr[:, :, ts(t, 512)])
                    ps_k = pp.tile([P, 512], f32, tag="psk", name="ps_k")
                    for co in range(cin_o):
                        nc.tensor.matmul(
                            ps_k[0:CQK, :], lhsT=wkT_s[:, co, :],
                            rhs=xf_bf[:, co, ts(t, 512)],
                            start=(co == 0), stop=(co == cin_o - 1))
                    nc.scalar.activation(
                        k_rep[0:CQK, t, :], ps_k[0:CQK, :], AF.Identity,
                        bias=bk_s[0:CQK, :])
                # q (nq columns only)
                for t in range(nq_t):
                    eng = nc.gpsimd if t % 2 == 0 else nc.sync
                    eng.dma_start(out=xq_bf[:, :, ts(t, 512)],
                                  in_=xqr[:, :, ts(t, 512)])
                    ps_q = pp.tile([P, 512], f32, tag="psk", name="ps_q")
                    for co in range(cin_o):
                        nc.tensor.matmul(
                            ps_q[0:CQK, :], lhsT=wqT_s[:, co, :],
                            rhs=xq_bf[:, co, ts(t, 512)],
                            start=(co == 0), stop=(co == cin_o - 1))
                    nc.scalar.activation(
                        q_rep[0:CQK, t, :], ps_q[0:CQK, :], AF.Identity,
                        bias=bq_s[0:CQK, :])
                if use8:
                    for t in range(4):
                        eng = nc.gpsimd if t % 2 == 0 else nc.sync
                        eng.dma_start(out=xf8_s[:, :, ts(t, n // 4)],
                                      in_=xf8r[:, :, ts(t, n // 4)])
                # replicate k, q to partition groups 1..3 (SBUF->SBUF DMA)
                for j in range(1, 4):
                    nc.gpsimd.dma_start(out=k_rep[ds(32 * j, 32), :, :],
                                        in_=k_rep[0:32, :, :])
                    nc.gpsimd.dma_start(out=q_rep[ds(32 * j, 32), :, :],
                                        in_=q_rep[0:32, :, :])
                # vT[m, c] = sum_cin xf[cin, m] * wvT[cin, c]  (no bias:
                # gamma*bv is folded into xres on the host).  fp8 DoubleRow
                # contracts both cin halves in one matmul; copies to fp8
                # SBUF alternate ScalarE/VectorE, 2 m-tiles per PSUM pair.
                for mp in range(m_tiles // 2):
                    ps_v = pp.tile([P, 2, C], f32, tag="psv", name="ps_v")
                    dst = vT8 if dr_of(mp) else vT
                    for h in range(2):
                        if dr_of(mp):
                            nc.tensor.matmul(
                                ps_v[:, h, :],
                                lhsT=xf8_s[:, :, ts(2 * mp + h, P)],
                                rhs=wvT8_s[:], start=True, stop=True,
                                perf_mode=DR)
                        else:
                            for co in range(cin_o):
                                nc.tensor.matmul(
                                    ps_v[:, h, :],
                                    lhsT=xf_bf[:, co, ts(2 * mp + h, P)],
                                    rhs=wvT_s[:, co, :],
                                    start=(co == 0), stop=(co == cin_o - 1))
                    if mp % 2 == 0:
                        nc.scalar.copy(dst[:, ds(2 * mp, 2), 0:C], ps_v)
                    else:
                        nc.vector.tensor_copy(dst[:, ds(2 * mp, 2), 0:C], ps_v)

            # ---- attention ------------------------------------------------
            outr = out_d[:].rearrange("(t p) c -> p t c", p=P)
            with tc.tile_pool(name="stp", bufs=4, space="PSUM") as stp, \
                 tc.tile_pool(name="op", bufs=1, space="PSUM") as op:
                LA = 4    # S-matmul lookahead (key tiles) for sw pipelining
                n_kt = 2 * n_grp
                for sc in range(n_sc):
                    out_ps = [op.tile([P, C + 1], f32, tag=f"ops{qt}",
                                      name=f"out_ps{qt}")
                              for qt in range(4)]
                    sts = {}

                    def emit_s(kt):
                        st = stp.tile([P, 512], f32, tag="st", name="st")
                        sts[kt] = st
                        bnd = kt % 4
                        nc.tensor.matmul(
                            st,
                            lhsT=k_rep[ds(32 * bnd, 32), kt // 4,
                                       ts(kt % 4, P)],
                            rhs=q_rep[ds(32 * bnd, 32), sc, :],
                            start=True, stop=True,
                            tile_position=(32 * bnd, 0))

                    for kt in range(LA):
                        emit_s(kt)
                    for gp in range(n_grp):
                        dr = dr_of(gp)
                        pT = ptp.tile([P, 2, 512], fp8 if dr else bf16,
                                      tag="pt8" if dr else "pt16",
                                      name="pT", bufs=3)
                        # the pair's two exp halves run CONCURRENTLY on
                        # ScalarE (true exp) and VectorE (Schraudolph bits)
                        st0 = sts.pop(2 * gp)
                        st1 = sts.pop(2 * gp + 1)
                        nc.scalar.activation(pT[:, 0, :], st0, AF.Exp)
                        if dr:
                            nc.vector.tensor_scalar(
                                out=pT[:, 1, :].bitcast(u8), in0=st1,
                                scalar1=SCH_SCALE, scalar2=56.0 + SCH_C_ADJ,
                                op0=ALU.mult, op1=ALU.add)
                        else:
                            nc.vector.tensor_scalar(
                                out=pT[:, 1, :].bitcast(i16), in0=st1,
                                scalar1=SCH_SCALE16, scalar2=SCH_BIAS16,
                                op0=ALU.mult, op1=ALU.add)
                        for kt in (2 * gp + LA, 2 * gp + 1 + LA):
                            if kt < n_kt:
                                emit_s(kt)
                        for qt in range(4):
                            if dr:
                                nc.tensor.matmul(
                                    out_ps[qt],
                                    lhsT=pT[:, :, ts(qt, P)],
                                    rhs=vT8[:, ds(2 * gp, 2), :],
                                    start=(gp == 0),
                                    stop=(gp == n_grp - 1),
                                    perf_mode=DR)
                            else:
                                for jj in range(2):
                                    nc.tensor.matmul(
                                        out_ps[qt],
                                        lhsT=pT[:, jj, ts(qt, P)],
                                        rhs=vT[:, 2 * gp + jj, :],
                                        start=(gp == 0 and jj == 0),
                                        stop=(gp == n_grp - 1 and jj == 1))
                    # epilogue: out = psum[:, :C] / rowsum + xres
                    for qt in range(4):
                        rec = small.tile([P, 1], f32, tag="rec", name="rec")
                        nc.vector.reciprocal(rec, out_ps[qt][:, C:C + 1])
                        ot = ostage.tile([P, C], f32, tag="ot", name="ot")
                        nc.scalar.activation(ot, out_ps[qt][:, 0:C],
                                             AF.Identity, scale=rec)
                        nc.vector.tensor_add(ot, ot, xres_s[:, 4 * sc + qt, :])
                        nc.sync.dma_start(out=outr[:, 4 * sc + qt, :], in_=ot)
    nc.compile()
    return nc


_nc_cache = {}


def _get_graph(n=N, nq=QCHUNK):
    key = (n, nq)
    if key not in _nc_cache:
        _nc_cache[key] = build_graph(n, nq)
    return _nc_cache[key]


def _make_in_maps(x, wq, bq, wk, bk, wv, bv, gamma, n=N, nq=QCHUNK):
    import ml_dtypes
    bf = ml_dtypes.bfloat16
    e4 = ml_dtypes.float8_e4m3
    xf = np.ascontiguousarray(x.reshape(B, C, n)).astype(np.float32)
    xf16 = xf.astype(bf)
    xf8 = xf.astype(e4)
    g = float(np.asarray(gamma).reshape(-1)[0])
    wqT = np.ascontiguousarray(np.asarray(wq, dtype=np.float32).T).astype(bf)
    wkT = np.ascontiguousarray(np.asarray(wk, dtype=np.float32).T).astype(bf)
    wvTf = np.ascontiguousarray((g * np.asarray(wv, dtype=np.float32)).T)
    wvT = wvTf.astype(bf)
    wvT8 = wvTf.astype(e4)
    bq2 = np.asarray(bq, dtype=np.float32).reshape(CQK, 1)
    bk2 = np.asarray(bk, dtype=np.float32).reshape(CQK, 1)
    gbv = (g * np.asarray(bv, dtype=np.float32))[None, :]
    nchunks = n // nq
    in_maps = []
    for i in range(NCORES):
        b, c = divmod(i, nchunks)
        n0 = c * nq
        xres = (xf[b].reshape(-1)[n0 * C:(n0 + nq) * C]
                .reshape(nq, C) + gbv).astype(np.float32)
        im = {
            "xf": xf16[b],
            "xq": np.ascontiguousarray(xf16[b][:, n0:n0 + nq]),
            "xres": xres,
            "wqT": wqT, "wkT": wkT,
            "bq": bq2, "bk": bk2,
        }
        if not AV_FP8:
            im["wvT"] = wvT
        if AV_FP8 or DR_MIX:
            im["xf8"] = xf8[b]
            im["wvT8"] = wvT8
        in_maps.append(im)
    return in_maps


def _assemble(results, n=N, nq=QCHUNK):
    nchunks = n // nq
    outs = []
    for b in range(B):
        buf = np.concatenate(
            [results[b * nchunks + c]["out"] for c in range(nchunks)], axis=0)
        outs.append(buf.reshape(C, Dd, Hh, Ww))
    return np.stack(outs).astype(np.float32)


def kernel(x, wq, bq, wk, bk, wv, bv, gamma):
    from concourse.bass_utils import run_bass_kernel_spmd
    nc = _get_graph()
    in_maps = _make_in_maps(x, wq, bq, wk, bk, wv, bv, gamma)
    res = run_bass_kernel_spmd(nc, in_maps, core_ids=list(range(NCORES)))
    return _assemble(res.results)
